# revision 108
# baseline (speedup 1.0000x reference)
"""Trainium2 Bass kernel for CNNLayer: conv(K=3 along H) + bias + tanh + topk(50) along H.

Full input contract:
  x:      [1024, 1, 200, 32] f32
  conv_w: [16, 1, 3, 1]      f32
  conv_b: [16]               f32
Output: [1024, 16, 50, 32] f32 (top-50 along H, sorted descending, after tanh)

Strategy: data-parallel over batch across 8 cores (128 samples/core).
Per sample:
  - load x[s]=[200,32] pre-transposed with 4 per-wblock DMAs straight
    from HBM into x8[w8, (wblock, h)] (partition stride 1 in HBM)
  - conv z[(o,w8), h] per wblock: 3 accumulating K=8 PE matmuls (one per
    tap) with block-diagonal weights; shifted column ranges handle the
    'same' padding exactly; wb pairs share one [128,2H] PSUM tile so
    eviction is 2 batched ACT Relu ops
  - PSUM eviction via ACT Relu(z + C_o) (tanh is monotonic so topk
    commutes past bias+tanh; the per-channel shift C_o > max|z| makes
    all candidates strictly positive so masked entries sort below)
  - top-56 via 7 rounds of DVE max8. The 24 inter-round maskings are
    split three ways to saturate all engines: 3 on DVE match_replace
    (exact-8 removal), 21 as ACT Sign(v_last - z) masks (-1/0/+1, made
    exact by mantissa bit-stuffing) applied by ONE batched Pool
    tensor_tensor multiply per round. DVE/ACT/Pool all run at 92-97%.
  - one batched strided ACT tanh(top50 + bias[o]) over all 4 wblocks
  - PE-transpose the result in two [128,100] halves to [(wb2,r),(o,w8)],
    ACT-evict to SBUF, and 4 per-wblock 3-dim DMAs scatter to HBM
Software pipeline: 15 stages/sample (conv, max0, [mask r, max r] x6,
out), emitted diagonally over ~15 in-flight samples.
"""

import os
import sys

for _p in ("/opt/trn_rl_repo", "/root/.axon_site/_ro/trn_rl_repo"):
    if os.path.isdir(_p) and _p not in sys.path:
        sys.path.insert(0, _p)

import numpy as np

N_CORES = 8
B, H, W = 1024, 200, 32
COUT, KH, TOPK = 16, 3, 50
BS = B // N_CORES  # samples per core
NWB = W // 8       # 4 w-blocks of 8 -> (o, w8) = 128 partition rows
NV = 56            # values extracted per row (7 rounds of 8)

_CACHE = {}

# Masking engine assignment. Each of the 24 (round 1-6, wb 0-3) maskings
# is done one of two ways:
#   - (r, wb) in MR_SLOTS: DVE match_replace (exact-8 removal)
#   - otherwise: ACT mask m = Sign(v_last - z) (-1/0/+1; extracted
#     values are negated by the apply and sink below the positive live
#     set, the 8th maps to 0; bit-stuffed distinctness keeps survivors
#     strictly below v_last) plus ONE batched Pool multiply per round.
# MR slots sit at wb edges (0 or 3) so the apply range stays contiguous.
# (The neuronx-cc Pool engine only implements the mult/add TT ALU ops,
# so cheaper apply flavors — STT, copy_predicated, TT-min — are out.)
MR_SLOTS = ((3, 3), (4, 3), (6, 3))
POOL_CMP_SLOTS = ()
STUFF_ENGINE = "dve"
MR_FILL = -1e30
# apply the mask via a Pool-initiated (SWDGE) accumulating DMA instead
# of a Pool tensor_tensor multiply. DISABLED: neuronx-cc's BIR verifier
# only accepts accum_op=add on DMACopy (probe_dma.py), and an additive
# mask can only be produced by a Pool 2-op tensor_scalar, which costs
# more than the TT multiply it would replace.
DMA_APPLY = False
# dummy PE transposes at t=0: keep the Tensor engine continuously busy
# through its ~3us p-state ramp so sample 0's conv runs at full clock
# (the cost model clocks a cold PE at 0.65GHz vs 2.4GHz warm)
PE_WARMUP = 8


def build_module(n_samples=BS, bufs=None, mr_slots=MR_SLOTS,
                 pool_cmp_slots=POOL_CMP_SLOTS, stuff_engine=STUFF_ENGINE,
                 dma_apply=DMA_APPLY, pe_warmup=PE_WARMUP):
    import concourse.bass as bass  # noqa: F401
    import concourse.tile as tile
    from concourse import bacc, mybir

    _bufs = dict(xt3=12, zpsum=3, zs=16, v=16, res=8,
                 otpsum=2, u=6, mk=8)
    _bufs.update(bufs or {})
    bufs = _bufs
    f32 = mybir.dt.float32
    u32 = mybir.dt.uint32
    nc = bacc.Bacc("TRN2", target_bir_lowering=False, debug=False,
                   num_devices=N_CORES)

    x = nc.dram_tensor("x", [n_samples, H, W], f32, kind="ExternalInput").ap()
    wkj = nc.dram_tensor("wkj", [8, KH * 128], f32, kind="ExternalInput").ap()
    # bias and shift packed as [128, 2]; hpat carries maskc in column 800:
    # fewer const DMAs -> fewer serialized HWDGE slots on the ramp
    bs = nc.dram_tensor("bs_p", [128, 2], f32, kind="ExternalInput").ap()
    ident = nc.dram_tensor("ident", [128, 128], f32, kind="ExternalInput").ap()
    hpat = nc.dram_tensor("hpatm", [128, NWB * H + 1], u32,
                          kind="ExternalInput").ap()
    out = nc.dram_tensor("out", [n_samples, COUT, TOPK, W], f32,
                         kind="ExternalOutput").ap()

    with tile.TileContext(nc) as tc:
        with (
            tc.tile_pool(name="const", bufs=1) as constp,
            tc.tile_pool(name="xt3", bufs=bufs["xt3"]) as xt3p,
            tc.tile_pool(name="zpsum", bufs=bufs["zpsum"], space="PSUM") as zpsum,
            tc.tile_pool(name="zs", bufs=bufs["zs"]) as zsp,
            tc.tile_pool(name="v", bufs=bufs["v"]) as vp,
            tc.tile_pool(name="res", bufs=bufs["res"]) as resp,
            tc.tile_pool(name="otpsum", bufs=bufs["otpsum"], space="PSUM") as otpsum,
            tc.tile_pool(name="u", bufs=bufs["u"]) as up,
            tc.tile_pool(name="mk", bufs=bufs["mk"]) as mkp,
            tc.tile_pool(name="warm", bufs=1, space="PSUM") as warmp,
        ):
            wkt = constp.tile([8, KH * 128], f32, tag="wk")
            nc.sync.dma_start(wkt[:], wkj[:])
            wk_sb = [wkt[:, 128 * k:128 * (k + 1)] for k in range(KH)]
            ident_sb = constp.tile([128, 128], f32)
            nc.sync.dma_start(ident_sb[:], ident[:])
            bs_sb = constp.tile([128, 2], f32)
            nc.sync.dma_start(bs_sb[:], bs[:])
            bias_sb = bs_sb[:, 0:1]
            shift_sb = bs_sb[:, 1:2]
            hpatm_sb = constp.tile([128, NWB * H + 1], u32)
            nc.sync.dma_start(hpatm_sb[:], hpat[:])
            hpat_sb = hpatm_sb[:, 0:NWB * H]
            maskc_sb = hpatm_sb[:, NWB * H:NWB * H + 1]

            if pe_warmup:
                warm = warmp.tile([128, 128], f32, tag="warm")
                for _ in range(pe_warmup):
                    nc.tensor.transpose(warm[:], ident_sb[:, :],
                                        ident_sb[:, :])

            state = {}

            def st_conv(s):
                # load x[s]=[200,32] directly transposed: one DMA per wb
                # pulls x8[w8, h] = x[h, 8wb+w8] (partition stride 1 in
                # HBM, free stride 32 — a legal 3-dim DMA AP). Costs
                # ~500ns/DMA on the underused DMA track but saves the
                # PE transposes and an ACT PSUM->SBUF copy.
                x8 = xt3p.tile([8, NWB * H], f32)
                for wb in range(NWB):
                    nc.sync.dma_start(
                        x8[:, H * wb:H * (wb + 1)],
                        x[s, :, 8 * wb:8 * wb + 8]
                        .rearrange("h w8 -> w8 h"),
                    )

                # conv per wb: 3 accumulating matmuls (taps), K=8;
                # per-tap column ranges make the H-boundary exact (no pad).
                # wb pairs share one [128, 2H] PSUM tile so eviction is 2
                # batched ACT ops instead of 4
                zs = zsp.tile([128, NWB * H], f32)
                for wb in range(NWB):
                    if wb % 2 == 0:
                        zpair = zpsum.tile([128, 2 * H], f32)
                    z = zpair[:, (wb % 2) * H:(wb % 2 + 1) * H]
                    xv = x8[:, H * wb:H * (wb + 1)]
                    nc.tensor.matmul(z[:, 0:H], wk_sb[1],
                                     xv[:, 0:H], start=True, stop=False)
                    nc.tensor.matmul(z[:, 1:H], wk_sb[0],
                                     xv[:, 0:H - 1], start=False, stop=False)
                    nc.tensor.matmul(z[:, 0:H - 1], wk_sb[2],
                                     xv[:, 1:H], start=False, stop=True)
                    # eviction as Relu(z + C_o): the per-channel shift C_o
                    # exceeds max|z| so nothing clamps and all candidates
                    # are strictly positive — threshold masking can then
                    # never collide with a live candidate. The tanh bias
                    # is b_o - C_o so the output is exact.
                    if wb % 2 == 1:
                        nc.scalar.activation(
                            zs[:, H * (wb - 1):H * (wb + 1)], zpair[:],
                            mybir.ActivationFunctionType.Relu,
                            bias=shift_sb)
                # bit-stuff the low 8 mantissa bits with the column index:
                # zs = (zs & 0xFFFFFF00) | h. Makes every value in a row
                # strictly distinct (distortion <= v*3e-5), so the Sign
                # threshold masking hits exactly the extracted instances —
                # it is otherwise broken by exact f32 duplicates at round
                # boundaries (PE fp32 conv produces them at ~1e-4/row)
                zu = zs[:].bitcast(u32)
                stuff_eng = nc.gpsimd if stuff_engine == "pool" else nc.vector
                stuff_eng.scalar_tensor_tensor(
                    zu, zu, maskc_sb, hpat_sb,
                    mybir.AluOpType.bitwise_and, mybir.AluOpType.bitwise_or)
                vt = vp.tile([128, NWB * NV], f32, tag="vt")
                state[s] = (zs, vt)

            def st_mask(s, r):
                zs, vt = state[s]
                zsl = [zs[:, H * wb:H * (wb + 1)] for wb in range(NWB)]
                other = [wb for wb in range(NWB) if (r, wb) not in mr_slots]
                for wb in range(NWB):
                    if (r, wb) in mr_slots:
                        # exact-8 removal on the DVE
                        nc.vector.match_replace(
                            zsl[wb],
                            vt[:, wb * NV + 8 * r - 8:wb * NV + 8 * r],
                            zsl[wb], MR_FILL)
                if other:
                    m = mkp.tile([128, NWB * H], f32, tag="mk")
                    for wb in other:
                        vlast = vt[:, wb * NV + 8 * r - 1:wb * NV + 8 * r]
                        if (r, wb) in pool_cmp_slots:
                            # 0/1 mask on Pool: is_lt(z, v_last)
                            nc.gpsimd.tensor_scalar(
                                m[:, H * wb:H * (wb + 1)], zsl[wb],
                                vlast, None, mybir.AluOpType.is_lt)
                        else:
                            # -1/0/+1 mask on ACT: Sign(v_last - z)
                            nc.scalar.activation(
                                m[:, H * wb:H * (wb + 1)], zsl[wb],
                                mybir.ActivationFunctionType.Sign,
                                bias=vlast, scale=-1.0)
                    lo, hi = min(other), max(other)
                    if dma_apply:
                        nc.gpsimd.dma_start(
                            zs[:, H * lo:H * (hi + 1)],
                            m[:, H * lo:H * (hi + 1)],
                            accum_op=mybir.AluOpType.mult)
                    else:
                        nc.gpsimd.tensor_tensor(
                            zs[:, H * lo:H * (hi + 1)],
                            m[:, H * lo:H * (hi + 1)],
                            zs[:, H * lo:H * (hi + 1)],
                            mybir.AluOpType.mult)

            def st_max(s, r):
                zs, vt = state[s]
                for wb in range(NWB):
                    nc.vector.max(vt[:, wb * NV + 8 * r:wb * NV + 8 * r + 8],
                                  zs[:, H * wb:H * (wb + 1)])

            def st_out(s):
                _, vt = state.pop(s)
                # one batched tanh(top50 + bias) over all 4 wb via a
                # strided view of vt (stride NV=56, take 50 per block)
                res = resp.tile([128, NWB * TOPK], f32, tag="res")
                nc.scalar.activation(
                    res[:].rearrange("p (wb r) -> p wb r", wb=NWB),
                    vt[:].rearrange("p (wb v) -> p wb v", wb=NWB)[:, :, 0:TOPK],
                    mybir.ActivationFunctionType.Tanh,
                    bias=bias_sb)
                # transpose [(o,w8), (wb2,r)] -> [(wb2,r), (o,w8)] in two
                # 100-column halves written side by side into ONE PSUM
                # tile, so the PSUM->SBUF eviction is a single ACT copy;
                # then per-wb DMA (DRAM side of one wb is a 3-dim AP)
                oT = otpsum.tile([100, 256], f32, tag="oT")
                nc.tensor.transpose(oT[:, 0:128], res[:, 0:100],
                                    ident_sb[:, :])
                nc.tensor.transpose(oT[:, 128:256], res[:, 100:200],
                                    ident_sb[:, :])
                u2 = up.tile([100, 256], f32, tag="u2")
                nc.scalar.copy(u2[:], oT[:])
                for hh in range(2):
                    for j in range(2):
                        wb = 2 * hh + j
                        nc.sync.dma_start(
                            out[s, :, :, 8 * wb:8 * wb + 8]
                            .rearrange("o r w8 -> r o w8"),
                            u2[50 * j:50 * j + 50, 128 * hh:128 * hh + 128]
                            .rearrange("r (o w8) -> r o w8", o=COUT),
                        )

            # software pipeline: 15 stages per sample (conv, max0,
            # [mask r, max r] for r=1..6, output), emitted diagonally so
            # each engine's in-order stream interleaves ~15 samples and
            # every cross-engine mask hop gets a full slot of slack
            stages = ([st_conv, lambda s: st_max(s, 0)] + [
                st for r in range(1, 7) for st in (
                    (lambda rr: (lambda s: st_mask(s, rr)))(r),
                    (lambda rr: (lambda s: st_max(s, rr)))(r),
                )
            ] + [st_out])
            nst = len(stages)
            for t in range(n_samples + nst - 1):
                for k in reversed(range(nst)):
                    s = t - k
                    if 0 <= s < n_samples:
                        stages[k](s)

    nc.compile()
    return nc


def _prep_consts(conv_w, conv_b):
    conv_w = np.asarray(conv_w, dtype=np.float32)
    conv_b = np.asarray(conv_b, dtype=np.float32)
    wmat = conv_w[:, 0, :, 0]  # [COUT, KH]
    wkj = np.zeros((KH, 8, 128), dtype=np.float32)
    for k in range(KH):
        for o in range(COUT):
            for w8 in range(8):
                wkj[k, w8, o * 8 + w8] = wmat[o, k]
    # per-channel positivity shift: C_o > max|z| guarantees z + C_o > 0
    # (|z| <= ||w_o||_1 * max|x|, and max|x| < 8 for any gaussian input)
    c_o = 8.0 * np.abs(wmat).sum(axis=1) + 1e-3  # [COUT]
    shift_p = np.repeat(c_o, 8).astype(np.float32)[:, None]  # [(o,w8), 1]
    bias_p = np.repeat(conv_b - c_o, 8).astype(np.float32)[:, None]
    ident = np.eye(128, dtype=np.float32)
    # bias/shift packed [128,2]; hpat with the stuff mask in column 800;
    # the 3 weight taps side by side in the free dim ([8, 3*128]) so one
    # DMA loads them and each tap slice starts at partition base 0
    bs_p = np.concatenate([bias_p, shift_p], axis=1).astype(np.float32)
    hpatm = np.empty((128, NWB * H + 1), dtype=np.uint32)
    hpatm[:, :NWB * H] = (np.arange(NWB * H, dtype=np.uint32) % H)[None, :]
    hpatm[:, NWB * H] = 0xFFFFFF00
    wk8 = wkj.transpose(1, 0, 2).reshape(8, KH * 128)
    return np.ascontiguousarray(wk8), bs_p, ident, hpatm


def get_compiled(n_samples=BS, mr_slots=MR_SLOTS,
                 pool_cmp_slots=POOL_CMP_SLOTS, stuff_engine=STUFF_ENGINE,
                 dma_apply=DMA_APPLY):
    key = (n_samples, tuple(mr_slots), tuple(pool_cmp_slots), stuff_engine,
           dma_apply)
    if key not in _CACHE:
        _CACHE[key] = build_module(n_samples, mr_slots=mr_slots,
                                   pool_cmp_slots=pool_cmp_slots,
                                   stuff_engine=stuff_engine,
                                   dma_apply=dma_apply)
    return _CACHE[key]


def _make_runner(nc):
    """Build a reusable jitted SPMD executor (jit traced once, reused across
    kernel() calls — run_bass_kernel_spmd re-traces on every call)."""
    import jax
    from jax.sharding import Mesh, PartitionSpec
    from jax.experimental.shard_map import shard_map
    from concourse import mybir
    from concourse.bass2jax import (_bass_exec_p, install_neuronx_cc_hook,
                                    partition_id_tensor)

    install_neuronx_cc_hook()
    in_names, out_names, out_avals, out_shapes = [], [], [], []
    pid = nc.partition_id_tensor.name if nc.partition_id_tensor else None
    for alloc in nc.m.functions[0].allocations:
        if not isinstance(alloc, mybir.MemoryLocationSet):
            continue
        name = alloc.memorylocations[0].name
        if alloc.kind == "ExternalInput":
            if name != pid:
                in_names.append(name)
        elif alloc.kind == "ExternalOutput":
            out_names.append(name)
            shape = tuple(alloc.tensor_shape)
            dtype = mybir.dt.np(alloc.dtype)
            out_avals.append(jax.core.ShapedArray(shape, dtype))
            out_shapes.append((shape, dtype))
    n_params = len(in_names)
    all_in = in_names + out_names + ([pid] if pid else [])

    def _body(*args):
        operands = list(args)
        if pid is not None:
            operands.append(partition_id_tensor())
        return tuple(_bass_exec_p.bind(
            *operands, out_avals=tuple(out_avals), in_names=tuple(all_in),
            out_names=tuple(out_names), lowering_input_output_aliases=(),
            sim_require_finite=True, sim_require_nnan=True, nc=nc))

    devices = jax.devices()[:N_CORES]
    assert len(devices) == N_CORES
    mesh = Mesh(np.asarray(devices), ("core",))
    nio = n_params + len(out_names)
    sharded = jax.jit(
        shard_map(_body, mesh=mesh,
                  in_specs=(PartitionSpec("core"),) * nio,
                  out_specs=(PartitionSpec("core"),) * len(out_names),
                  check_rep=False),
        donate_argnums=tuple(range(n_params, nio)), keep_unused=True)

    import jax.numpy as jnp
    from jax.sharding import NamedSharding
    shd = NamedSharding(mesh, PartitionSpec("core"))
    make_zeros = jax.jit(
        lambda: tuple(jnp.zeros((N_CORES * s[0],) + s[1:], d)
                      for s, d in out_shapes),
        out_shardings=(shd,) * len(out_shapes))

    def run(global_inputs):
        ins = [global_inputs[n] for n in in_names]
        zeros = jax.block_until_ready(make_zeros())
        outs = jax.block_until_ready(sharded(*ins, *zeros))
        return {n: np.asarray(o) for n, o in zip(out_names, outs)}

    return run


def kernel(x, conv_w, conv_b):
    x = np.asarray(x, dtype=np.float32)
    nc = get_compiled(BS)
    wkj, bs_p, ident, hpatm = _prep_consts(conv_w, conv_b)
    xs = np.ascontiguousarray(x.reshape(B, H, W))  # squeeze CIN=1

    if "runner" not in _CACHE:
        try:
            _CACHE["runner"] = _make_runner(nc)
        except Exception:
            _CACHE["runner"] = None
    runner = _CACHE["runner"]

    if runner is not None:
        global_inputs = {
            "x": xs,
            "wkj": np.concatenate([wkj] * N_CORES, axis=0),
            "bs_p": np.concatenate([bs_p] * N_CORES, axis=0),
            "ident": np.concatenate([ident] * N_CORES, axis=0),
            "hpatm": np.concatenate([hpatm] * N_CORES, axis=0),
        }
        # the axon terminal occasionally throws a transient
        # NRT_EXEC_UNIT_UNRECOVERABLE; a retry on a fresh executable
        # succeeds, so retry before giving up on the fast path
        for attempt in range(3):
            try:
                out = runner(global_inputs)["out"]
                return out.reshape(B, COUT, TOPK, W)
            except Exception:
                import time as _time
                _time.sleep(2.0 * (attempt + 1))
                try:
                    runner = _make_runner(nc)
                    _CACHE["runner"] = runner
                except Exception:
                    break

    # fallback: stock SPMD path (re-traces jit per call)
    from concourse.bass_utils import run_bass_kernel_spmd
    in_maps = []
    for c in range(N_CORES):
        in_maps.append({
            "x": np.ascontiguousarray(xs[c * BS:(c + 1) * BS]),
            "wkj": wkj,
            "bs_p": bs_p,
            "ident": ident,
            "hpatm": hpatm,
        })
    last_err = None
    for attempt in range(3):
        try:
            res = run_bass_kernel_spmd(nc, in_maps, list(range(N_CORES)))
            return np.concatenate(
                [res.results[c]["out"] for c in range(N_CORES)], axis=0)
        except Exception as e:
            last_err = e
            import time as _time
            _time.sleep(2.0 * (attempt + 1))
    raise last_err



# revision 109
# speedup vs baseline: 1.0022x; 1.0022x over previous
"""Trainium2 Bass kernel for CNNLayer: conv(K=3 along H) + bias + tanh + topk(50) along H.

Full input contract:
  x:      [1024, 1, 200, 32] f32
  conv_w: [16, 1, 3, 1]      f32
  conv_b: [16]               f32
Output: [1024, 16, 50, 32] f32 (top-50 along H, sorted descending, after tanh)

Strategy: data-parallel over batch across 8 cores (128 samples/core).
Per sample:
  - load x[s]=[200,32] pre-transposed with 4 per-wblock DMAs straight
    from HBM into x8[w8, (wblock, h)] (partition stride 1 in HBM)
  - conv z[(o,w8), h] per wblock: 3 accumulating K=8 PE matmuls (one per
    tap) with block-diagonal weights; shifted column ranges handle the
    'same' padding exactly; wb pairs share one [128,2H] PSUM tile so
    eviction is 2 batched ACT Relu ops
  - PSUM eviction via ACT Relu(z + C_o) (tanh is monotonic so topk
    commutes past bias+tanh; the per-channel shift C_o > max|z| makes
    all candidates strictly positive so masked entries sort below)
  - top-56 via 7 rounds of DVE max8. The 24 inter-round maskings are
    split three ways to saturate all engines: 3 on DVE match_replace
    (exact-8 removal), 21 as ACT Sign(v_last - z) masks (-1/0/+1, made
    exact by mantissa bit-stuffing) applied by ONE batched Pool
    tensor_tensor multiply per round. DVE/ACT/Pool all run at 92-97%.
  - one batched strided ACT tanh(top50 + bias[o]) over all 4 wblocks
  - PE-transpose the result in two [128,100] halves to [(wb2,r),(o,w8)],
    ACT-evict to SBUF, and 4 per-wblock 3-dim DMAs scatter to HBM
Software pipeline: 15 stages/sample (conv, max0, [mask r, max r] x6,
out), emitted diagonally over ~15 in-flight samples.
"""

import os
import sys

for _p in ("/opt/trn_rl_repo", "/root/.axon_site/_ro/trn_rl_repo"):
    if os.path.isdir(_p) and _p not in sys.path:
        sys.path.insert(0, _p)

import numpy as np

N_CORES = 8
B, H, W = 1024, 200, 32
COUT, KH, TOPK = 16, 3, 50
BS = B // N_CORES  # samples per core
NWB = W // 8       # 4 w-blocks of 8 -> (o, w8) = 128 partition rows
NV = 56            # values extracted per row (7 rounds of 8)

_CACHE = {}

# Masking engine assignment. Each of the 24 (round 1-6, wb 0-3) maskings
# is done one of two ways:
#   - (r, wb) in MR_SLOTS: DVE match_replace (exact-8 removal)
#   - otherwise: ACT mask m = Sign(v_last - z) (-1/0/+1; extracted
#     values are negated by the apply and sink below the positive live
#     set, the 8th maps to 0; bit-stuffed distinctness keeps survivors
#     strictly below v_last) plus ONE batched Pool multiply per round.
# MR slots sit at wb edges (0 or 3) so the apply range stays contiguous.
# (The neuronx-cc Pool engine only implements the mult/add TT ALU ops,
# so cheaper apply flavors — STT, copy_predicated, TT-min — are out.)
MR_SLOTS = ((3, 3), (4, 3), (6, 3))
POOL_CMP_SLOTS = ()
STUFF_ENGINE = "dve"
MR_FILL = -1e30
# apply the mask via a Pool-initiated (SWDGE) accumulating DMA instead
# of a Pool tensor_tensor multiply. DISABLED: neuronx-cc's BIR verifier
# only accepts accum_op=add on DMACopy (probe_dma.py), and an additive
# mask can only be produced by a Pool 2-op tensor_scalar, which costs
# more than the TT multiply it would replace.
DMA_APPLY = False
# dummy PE transposes at t=0: keep the Tensor engine continuously busy
# through its ~3us p-state ramp so sample 0's conv runs at full clock
# (the cost model clocks a cold PE at 0.65GHz vs 2.4GHz warm)
PE_WARMUP = 8


def build_module(n_samples=BS, bufs=None, mr_slots=MR_SLOTS,
                 pool_cmp_slots=POOL_CMP_SLOTS, stuff_engine=STUFF_ENGINE,
                 dma_apply=DMA_APPLY, pe_warmup=PE_WARMUP):
    import concourse.bass as bass  # noqa: F401
    import concourse.tile as tile
    from concourse import bacc, mybir

    _bufs = dict(xt3=12, zpsum=3, zs=16, v=16, res=8,
                 otpsum=2, u=6, mk=8)
    _bufs.update(bufs or {})
    bufs = _bufs
    f32 = mybir.dt.float32
    u32 = mybir.dt.uint32
    nc = bacc.Bacc("TRN2", target_bir_lowering=False, debug=False,
                   num_devices=N_CORES)

    x = nc.dram_tensor("x", [n_samples, H, W], f32, kind="ExternalInput").ap()
    wkj = nc.dram_tensor("wkj", [8, KH * 128], f32, kind="ExternalInput").ap()
    # bias and shift packed as [128, 2]; hpat carries maskc in column 800:
    # fewer const DMAs -> fewer serialized HWDGE slots on the ramp
    bs = nc.dram_tensor("bs_p", [128, 2], f32, kind="ExternalInput").ap()
    ident = nc.dram_tensor("ident", [128, 128], f32, kind="ExternalInput").ap()
    hpat = nc.dram_tensor("hpatm", [128, NWB * H + 1], u32,
                          kind="ExternalInput").ap()
    out = nc.dram_tensor("out", [n_samples, COUT, TOPK, W], f32,
                         kind="ExternalOutput").ap()

    with tile.TileContext(nc) as tc:
        with (
            tc.tile_pool(name="const", bufs=1) as constp,
            tc.tile_pool(name="xt3", bufs=bufs["xt3"]) as xt3p,
            tc.tile_pool(name="zpsum", bufs=bufs["zpsum"], space="PSUM") as zpsum,
            tc.tile_pool(name="zs", bufs=bufs["zs"]) as zsp,
            tc.tile_pool(name="v", bufs=bufs["v"]) as vp,
            tc.tile_pool(name="res", bufs=bufs["res"]) as resp,
            tc.tile_pool(name="otpsum", bufs=bufs["otpsum"], space="PSUM") as otpsum,
            tc.tile_pool(name="u", bufs=bufs["u"]) as up,
            tc.tile_pool(name="mk", bufs=bufs["mk"]) as mkp,
            tc.tile_pool(name="warm", bufs=1, space="PSUM") as warmp,
        ):
            # ident first: the PE warmup below only needs ident, so the
            # p-state ramp starts as early as possible
            ident_sb = constp.tile([128, 128], f32)
            nc.sync.dma_start(ident_sb[:], ident[:])
            wkt = constp.tile([8, KH * 128], f32, tag="wk")
            nc.sync.dma_start(wkt[:], wkj[:])
            wk_sb = [wkt[:, 128 * k:128 * (k + 1)] for k in range(KH)]
            bs_sb = constp.tile([128, 2], f32)
            nc.sync.dma_start(bs_sb[:], bs[:])
            bias_sb = bs_sb[:, 0:1]
            shift_sb = bs_sb[:, 1:2]
            hpatm_sb = constp.tile([128, NWB * H + 1], u32)
            nc.sync.dma_start(hpatm_sb[:], hpat[:])
            hpat_sb = hpatm_sb[:, 0:NWB * H]
            maskc_sb = hpatm_sb[:, NWB * H:NWB * H + 1]

            if pe_warmup:
                warm = warmp.tile([128, 128], f32, tag="warm")
                for _ in range(pe_warmup):
                    nc.tensor.transpose(warm[:], ident_sb[:, :],
                                        ident_sb[:, :])

            state = {}

            def st_conv(s):
                # load x[s]=[200,32] directly transposed: one DMA per wb
                # pulls x8[w8, h] = x[h, 8wb+w8] (partition stride 1 in
                # HBM, free stride 32 — a legal 3-dim DMA AP). Costs
                # ~500ns/DMA on the underused DMA track but saves the
                # PE transposes and an ACT PSUM->SBUF copy.
                x8 = xt3p.tile([8, NWB * H], f32)
                for wb in range(NWB):
                    nc.sync.dma_start(
                        x8[:, H * wb:H * (wb + 1)],
                        x[s, :, 8 * wb:8 * wb + 8]
                        .rearrange("h w8 -> w8 h"),
                    )

                # conv per wb: 3 accumulating matmuls (taps), K=8;
                # per-tap column ranges make the H-boundary exact (no pad).
                # wb pairs share one [128, 2H] PSUM tile so eviction is 2
                # batched ACT ops instead of 4
                zs = zsp.tile([128, NWB * H], f32)
                for wb in range(NWB):
                    if wb % 2 == 0:
                        zpair = zpsum.tile([128, 2 * H], f32)
                    z = zpair[:, (wb % 2) * H:(wb % 2 + 1) * H]
                    xv = x8[:, H * wb:H * (wb + 1)]
                    nc.tensor.matmul(z[:, 0:H], wk_sb[1],
                                     xv[:, 0:H], start=True, stop=False)
                    nc.tensor.matmul(z[:, 1:H], wk_sb[0],
                                     xv[:, 0:H - 1], start=False, stop=False)
                    nc.tensor.matmul(z[:, 0:H - 1], wk_sb[2],
                                     xv[:, 1:H], start=False, stop=True)
                    # eviction as Relu(z + C_o): the per-channel shift C_o
                    # exceeds max|z| so nothing clamps and all candidates
                    # are strictly positive — threshold masking can then
                    # never collide with a live candidate. The tanh bias
                    # is b_o - C_o so the output is exact.
                    if wb % 2 == 1:
                        nc.scalar.activation(
                            zs[:, H * (wb - 1):H * (wb + 1)], zpair[:],
                            mybir.ActivationFunctionType.Relu,
                            bias=shift_sb)
                # bit-stuff the low 8 mantissa bits with the column index:
                # zs = (zs & 0xFFFFFF00) | h. Makes every value in a row
                # strictly distinct (distortion <= v*3e-5), so the Sign
                # threshold masking hits exactly the extracted instances —
                # it is otherwise broken by exact f32 duplicates at round
                # boundaries (PE fp32 conv produces them at ~1e-4/row)
                zu = zs[:].bitcast(u32)
                stuff_eng = nc.gpsimd if stuff_engine == "pool" else nc.vector
                stuff_eng.scalar_tensor_tensor(
                    zu, zu, maskc_sb, hpat_sb,
                    mybir.AluOpType.bitwise_and, mybir.AluOpType.bitwise_or)
                vt = vp.tile([128, NWB * NV], f32, tag="vt")
                state[s] = (zs, vt)

            def st_mask(s, r):
                zs, vt = state[s]
                zsl = [zs[:, H * wb:H * (wb + 1)] for wb in range(NWB)]
                other = [wb for wb in range(NWB) if (r, wb) not in mr_slots]
                for wb in range(NWB):
                    if (r, wb) in mr_slots:
                        # exact-8 removal on the DVE
                        nc.vector.match_replace(
                            zsl[wb],
                            vt[:, wb * NV + 8 * r - 8:wb * NV + 8 * r],
                            zsl[wb], MR_FILL)
                if other:
                    m = mkp.tile([128, NWB * H], f32, tag="mk")
                    for wb in other:
                        vlast = vt[:, wb * NV + 8 * r - 1:wb * NV + 8 * r]
                        if (r, wb) in pool_cmp_slots:
                            # 0/1 mask on Pool: is_lt(z, v_last)
                            nc.gpsimd.tensor_scalar(
                                m[:, H * wb:H * (wb + 1)], zsl[wb],
                                vlast, None, mybir.AluOpType.is_lt)
                        else:
                            # -1/0/+1 mask on ACT: Sign(v_last - z)
                            nc.scalar.activation(
                                m[:, H * wb:H * (wb + 1)], zsl[wb],
                                mybir.ActivationFunctionType.Sign,
                                bias=vlast, scale=-1.0)
                    lo, hi = min(other), max(other)
                    if dma_apply:
                        nc.gpsimd.dma_start(
                            zs[:, H * lo:H * (hi + 1)],
                            m[:, H * lo:H * (hi + 1)],
                            accum_op=mybir.AluOpType.mult)
                    else:
                        nc.gpsimd.tensor_tensor(
                            zs[:, H * lo:H * (hi + 1)],
                            m[:, H * lo:H * (hi + 1)],
                            zs[:, H * lo:H * (hi + 1)],
                            mybir.AluOpType.mult)

            def st_max(s, r):
                zs, vt = state[s]
                for wb in range(NWB):
                    nc.vector.max(vt[:, wb * NV + 8 * r:wb * NV + 8 * r + 8],
                                  zs[:, H * wb:H * (wb + 1)])

            def st_out(s):
                _, vt = state.pop(s)
                # one batched tanh(top50 + bias) over all 4 wb via a
                # strided view of vt (stride NV=56, take 50 per block)
                res = resp.tile([128, NWB * TOPK], f32, tag="res")
                nc.scalar.activation(
                    res[:].rearrange("p (wb r) -> p wb r", wb=NWB),
                    vt[:].rearrange("p (wb v) -> p wb v", wb=NWB)[:, :, 0:TOPK],
                    mybir.ActivationFunctionType.Tanh,
                    bias=bias_sb)
                # transpose [(o,w8), (wb2,r)] -> [(wb2,r), (o,w8)] in two
                # 100-column halves written side by side into ONE PSUM
                # tile, so the PSUM->SBUF eviction is a single ACT copy;
                # then per-wb DMA (DRAM side of one wb is a 3-dim AP)
                oT = otpsum.tile([100, 256], f32, tag="oT")
                nc.tensor.transpose(oT[:, 0:128], res[:, 0:100],
                                    ident_sb[:, :])
                nc.tensor.transpose(oT[:, 128:256], res[:, 100:200],
                                    ident_sb[:, :])
                u2 = up.tile([100, 256], f32, tag="u2")
                nc.scalar.copy(u2[:], oT[:])
                for hh in range(2):
                    for j in range(2):
                        wb = 2 * hh + j
                        nc.sync.dma_start(
                            out[s, :, :, 8 * wb:8 * wb + 8]
                            .rearrange("o r w8 -> r o w8"),
                            u2[50 * j:50 * j + 50, 128 * hh:128 * hh + 128]
                            .rearrange("r (o w8) -> r o w8", o=COUT),
                        )

            # software pipeline: 15 stages per sample (conv, max0,
            # [mask r, max r] for r=1..6, output), emitted diagonally so
            # each engine's in-order stream interleaves ~15 samples and
            # every cross-engine mask hop gets a full slot of slack
            stages = ([st_conv, lambda s: st_max(s, 0)] + [
                st for r in range(1, 7) for st in (
                    (lambda rr: (lambda s: st_mask(s, rr)))(r),
                    (lambda rr: (lambda s: st_max(s, rr)))(r),
                )
            ] + [st_out])
            nst = len(stages)
            for t in range(n_samples + nst - 1):
                for k in reversed(range(nst)):
                    s = t - k
                    if 0 <= s < n_samples:
                        stages[k](s)

    nc.compile()
    return nc


def _prep_consts(conv_w, conv_b):
    conv_w = np.asarray(conv_w, dtype=np.float32)
    conv_b = np.asarray(conv_b, dtype=np.float32)
    wmat = conv_w[:, 0, :, 0]  # [COUT, KH]
    wkj = np.zeros((KH, 8, 128), dtype=np.float32)
    for k in range(KH):
        for o in range(COUT):
            for w8 in range(8):
                wkj[k, w8, o * 8 + w8] = wmat[o, k]
    # per-channel positivity shift: C_o > max|z| guarantees z + C_o > 0
    # (|z| <= ||w_o||_1 * max|x|, and max|x| < 8 for any gaussian input)
    c_o = 8.0 * np.abs(wmat).sum(axis=1) + 1e-3  # [COUT]
    shift_p = np.repeat(c_o, 8).astype(np.float32)[:, None]  # [(o,w8), 1]
    bias_p = np.repeat(conv_b - c_o, 8).astype(np.float32)[:, None]
    ident = np.eye(128, dtype=np.float32)
    # bias/shift packed [128,2]; hpat with the stuff mask in column 800;
    # the 3 weight taps side by side in the free dim ([8, 3*128]) so one
    # DMA loads them and each tap slice starts at partition base 0
    bs_p = np.concatenate([bias_p, shift_p], axis=1).astype(np.float32)
    hpatm = np.empty((128, NWB * H + 1), dtype=np.uint32)
    hpatm[:, :NWB * H] = (np.arange(NWB * H, dtype=np.uint32) % H)[None, :]
    hpatm[:, NWB * H] = 0xFFFFFF00
    wk8 = wkj.transpose(1, 0, 2).reshape(8, KH * 128)
    return np.ascontiguousarray(wk8), bs_p, ident, hpatm


def get_compiled(n_samples=BS, mr_slots=MR_SLOTS,
                 pool_cmp_slots=POOL_CMP_SLOTS, stuff_engine=STUFF_ENGINE,
                 dma_apply=DMA_APPLY):
    key = (n_samples, tuple(mr_slots), tuple(pool_cmp_slots), stuff_engine,
           dma_apply)
    if key not in _CACHE:
        _CACHE[key] = build_module(n_samples, mr_slots=mr_slots,
                                   pool_cmp_slots=pool_cmp_slots,
                                   stuff_engine=stuff_engine,
                                   dma_apply=dma_apply)
    return _CACHE[key]


def _make_runner(nc):
    """Build a reusable jitted SPMD executor (jit traced once, reused across
    kernel() calls — run_bass_kernel_spmd re-traces on every call)."""
    import jax
    from jax.sharding import Mesh, PartitionSpec
    from jax.experimental.shard_map import shard_map
    from concourse import mybir
    from concourse.bass2jax import (_bass_exec_p, install_neuronx_cc_hook,
                                    partition_id_tensor)

    install_neuronx_cc_hook()
    in_names, out_names, out_avals, out_shapes = [], [], [], []
    pid = nc.partition_id_tensor.name if nc.partition_id_tensor else None
    for alloc in nc.m.functions[0].allocations:
        if not isinstance(alloc, mybir.MemoryLocationSet):
            continue
        name = alloc.memorylocations[0].name
        if alloc.kind == "ExternalInput":
            if name != pid:
                in_names.append(name)
        elif alloc.kind == "ExternalOutput":
            out_names.append(name)
            shape = tuple(alloc.tensor_shape)
            dtype = mybir.dt.np(alloc.dtype)
            out_avals.append(jax.core.ShapedArray(shape, dtype))
            out_shapes.append((shape, dtype))
    n_params = len(in_names)
    all_in = in_names + out_names + ([pid] if pid else [])

    def _body(*args):
        operands = list(args)
        if pid is not None:
            operands.append(partition_id_tensor())
        return tuple(_bass_exec_p.bind(
            *operands, out_avals=tuple(out_avals), in_names=tuple(all_in),
            out_names=tuple(out_names), lowering_input_output_aliases=(),
            sim_require_finite=True, sim_require_nnan=True, nc=nc))

    devices = jax.devices()[:N_CORES]
    assert len(devices) == N_CORES
    mesh = Mesh(np.asarray(devices), ("core",))
    nio = n_params + len(out_names)
    sharded = jax.jit(
        shard_map(_body, mesh=mesh,
                  in_specs=(PartitionSpec("core"),) * nio,
                  out_specs=(PartitionSpec("core"),) * len(out_names),
                  check_rep=False),
        donate_argnums=tuple(range(n_params, nio)), keep_unused=True)

    import jax.numpy as jnp
    from jax.sharding import NamedSharding
    shd = NamedSharding(mesh, PartitionSpec("core"))
    make_zeros = jax.jit(
        lambda: tuple(jnp.zeros((N_CORES * s[0],) + s[1:], d)
                      for s, d in out_shapes),
        out_shardings=(shd,) * len(out_shapes))

    def run(global_inputs):
        ins = [global_inputs[n] for n in in_names]
        zeros = jax.block_until_ready(make_zeros())
        outs = jax.block_until_ready(sharded(*ins, *zeros))
        return {n: np.asarray(o) for n, o in zip(out_names, outs)}

    return run


def kernel(x, conv_w, conv_b):
    x = np.asarray(x, dtype=np.float32)
    nc = get_compiled(BS)
    wkj, bs_p, ident, hpatm = _prep_consts(conv_w, conv_b)
    xs = np.ascontiguousarray(x.reshape(B, H, W))  # squeeze CIN=1

    if "runner" not in _CACHE:
        try:
            _CACHE["runner"] = _make_runner(nc)
        except Exception:
            _CACHE["runner"] = None
    runner = _CACHE["runner"]

    if runner is not None:
        global_inputs = {
            "x": xs,
            "wkj": np.concatenate([wkj] * N_CORES, axis=0),
            "bs_p": np.concatenate([bs_p] * N_CORES, axis=0),
            "ident": np.concatenate([ident] * N_CORES, axis=0),
            "hpatm": np.concatenate([hpatm] * N_CORES, axis=0),
        }
        # the axon terminal occasionally throws a transient
        # NRT_EXEC_UNIT_UNRECOVERABLE; a retry on a fresh executable
        # succeeds, so retry before giving up on the fast path
        for attempt in range(3):
            try:
                out = runner(global_inputs)["out"]
                return out.reshape(B, COUT, TOPK, W)
            except Exception:
                import time as _time
                _time.sleep(2.0 * (attempt + 1))
                try:
                    runner = _make_runner(nc)
                    _CACHE["runner"] = runner
                except Exception:
                    break

    # fallback: stock SPMD path (re-traces jit per call)
    from concourse.bass_utils import run_bass_kernel_spmd
    in_maps = []
    for c in range(N_CORES):
        in_maps.append({
            "x": np.ascontiguousarray(xs[c * BS:(c + 1) * BS]),
            "wkj": wkj,
            "bs_p": bs_p,
            "ident": ident,
            "hpatm": hpatm,
        })
    last_err = None
    for attempt in range(3):
        try:
            res = run_bass_kernel_spmd(nc, in_maps, list(range(N_CORES)))
            return np.concatenate(
                [res.results[c]["out"] for c in range(N_CORES)], axis=0)
        except Exception as e:
            last_err = e
            import time as _time
            _time.sleep(2.0 * (attempt + 1))
    raise last_err



# revision 111
# speedup vs baseline: 1.0036x; 1.0014x over previous
"""Trainium2 Bass kernel for CNNLayer: conv(K=3 along H) + bias + tanh + topk(50) along H.

Full input contract:
  x:      [1024, 1, 200, 32] f32
  conv_w: [16, 1, 3, 1]      f32
  conv_b: [16]               f32
Output: [1024, 16, 50, 32] f32 (top-50 along H, sorted descending, after tanh)

Strategy: data-parallel over batch across 8 cores (128 samples/core).
Per sample:
  - load x[s]=[200,32] pre-transposed with 4 per-wblock DMAs straight
    from HBM into x8[w8, (wblock, h)] (partition stride 1 in HBM)
  - conv z[(o,w8), h] per wblock: 3 accumulating K=8 PE matmuls (one per
    tap) with block-diagonal weights; shifted column ranges handle the
    'same' padding exactly; wb pairs share one [128,2H] PSUM tile so
    eviction is 2 batched ACT Relu ops
  - PSUM eviction via ACT Relu(z + C_o) (tanh is monotonic so topk
    commutes past bias+tanh; the per-channel shift C_o > max|z| makes
    all candidates strictly positive so masked entries sort below)
  - top-56 via 7 rounds of DVE max8. The 24 inter-round maskings are
    split three ways to saturate all engines: 3 on DVE match_replace
    (exact-8 removal), 21 as ACT Sign(v_last - z) masks (-1/0/+1, made
    exact by mantissa bit-stuffing) applied by ONE batched Pool
    tensor_tensor multiply per round. DVE/ACT/Pool all run at 92-97%.
  - one batched strided ACT tanh(top50 + bias[o]) over all 4 wblocks
  - PE-transpose the result in two [128,100] halves to [(wb2,r),(o,w8)],
    ACT-evict to SBUF, and 4 per-wblock 3-dim DMAs scatter to HBM
Software pipeline: 15 stages/sample (conv, max0, [mask r, max r] x6,
out), emitted diagonally over ~15 in-flight samples.
"""

import os
import sys

for _p in ("/opt/trn_rl_repo", "/root/.axon_site/_ro/trn_rl_repo"):
    if os.path.isdir(_p) and _p not in sys.path:
        sys.path.insert(0, _p)

import numpy as np

N_CORES = 8
B, H, W = 1024, 200, 32
COUT, KH, TOPK = 16, 3, 50
BS = B // N_CORES  # samples per core
NWB = W // 8       # 4 w-blocks of 8 -> (o, w8) = 128 partition rows
NV = 56            # values extracted per row (7 rounds of 8)

_CACHE = {}

# Masking engine assignment. Each of the 24 (round 1-6, wb 0-3) maskings
# is done one of two ways:
#   - (r, wb) in MR_SLOTS: DVE match_replace (exact-8 removal)
#   - otherwise: ACT mask m = Sign(v_last - z) (-1/0/+1; extracted
#     values are negated by the apply and sink below the positive live
#     set, the 8th maps to 0; bit-stuffed distinctness keeps survivors
#     strictly below v_last) plus ONE batched Pool multiply per round.
# MR slots sit at wb edges (0 or 3) so the apply range stays contiguous.
# (The neuronx-cc Pool engine only implements the mult/add TT ALU ops,
# so cheaper apply flavors — STT, copy_predicated, TT-min — are out.)
MR_SLOTS = ((3, 3), (4, 3), (6, 3))
POOL_CMP_SLOTS = ()
STUFF_ENGINE = "dve"
MR_FILL = -1e30
# apply the mask via a Pool-initiated (SWDGE) accumulating DMA instead
# of a Pool tensor_tensor multiply. DISABLED: neuronx-cc's BIR verifier
# only accepts accum_op=add on DMACopy (probe_dma.py), and an additive
# mask can only be produced by a Pool 2-op tensor_scalar, which costs
# more than the TT multiply it would replace.
DMA_APPLY = False
# dummy PE transposes at t=0: keep the Tensor engine continuously busy
# through its ~3us p-state ramp so sample 0's conv runs at full clock
# (the cost model clocks a cold PE at 0.65GHz vs 2.4GHz warm)
PE_WARMUP = 8


def build_module(n_samples=BS, bufs=None, mr_slots=MR_SLOTS,
                 pool_cmp_slots=POOL_CMP_SLOTS, stuff_engine=STUFF_ENGINE,
                 dma_apply=DMA_APPLY, pe_warmup=PE_WARMUP):
    import concourse.bass as bass  # noqa: F401
    import concourse.tile as tile
    from concourse import bacc, mybir

    _bufs = dict(xt3=12, zpsum=3, zs=16, v=16, res=8,
                 otpsum=2, u=6, mk=8)
    _bufs.update(bufs or {})
    bufs = _bufs
    f32 = mybir.dt.float32
    u32 = mybir.dt.uint32
    nc = bacc.Bacc("TRN2", target_bir_lowering=False, debug=False,
                   num_devices=N_CORES)

    x = nc.dram_tensor("x", [n_samples, H, W], f32, kind="ExternalInput").ap()
    wkj = nc.dram_tensor("wkj", [8, KH * 128], f32, kind="ExternalInput").ap()
    # bias and shift packed as [128, 2]; hpat carries maskc in column 800:
    # fewer const DMAs -> fewer serialized HWDGE slots on the ramp
    bs = nc.dram_tensor("bs_p", [128, 2], f32, kind="ExternalInput").ap()
    ident = nc.dram_tensor("ident", [128, 128], f32, kind="ExternalInput").ap()
    hpat = nc.dram_tensor("hpatm", [128, NWB * H + 1], u32,
                          kind="ExternalInput").ap()
    out = nc.dram_tensor("out", [n_samples, COUT, TOPK, W], f32,
                         kind="ExternalOutput").ap()

    with tile.TileContext(nc) as tc:
        with (
            tc.tile_pool(name="const", bufs=1) as constp,
            tc.tile_pool(name="xt3", bufs=bufs["xt3"]) as xt3p,
            tc.tile_pool(name="zpsum", bufs=bufs["zpsum"], space="PSUM") as zpsum,
            tc.tile_pool(name="zs", bufs=bufs["zs"]) as zsp,
            tc.tile_pool(name="v", bufs=bufs["v"]) as vp,
            tc.tile_pool(name="res", bufs=bufs["res"]) as resp,
            tc.tile_pool(name="otpsum", bufs=bufs["otpsum"], space="PSUM") as otpsum,
            tc.tile_pool(name="u", bufs=bufs["u"]) as up,
            tc.tile_pool(name="mk", bufs=bufs["mk"]) as mkp,
            tc.tile_pool(name="warm", bufs=1, space="PSUM") as warmp,
        ):
            # ident first: the PE warmup below only needs ident, so the
            # p-state ramp starts as early as possible
            ident_sb = constp.tile([128, 128], f32)
            nc.sync.dma_start(ident_sb[:], ident[:])
            wkt = constp.tile([8, KH * 128], f32, tag="wk")
            nc.sync.dma_start(wkt[:], wkj[:])
            wk_sb = [wkt[:, 128 * k:128 * (k + 1)] for k in range(KH)]
            # bs/hpatm tiles are declared here but their DMAs are deferred
            # into st_conv(0), AFTER sample 0's x-loads: the 1.4us hpatm
            # transfer otherwise serializes ahead of the x-loads on the
            # DMA engines and gates the whole pipeline ramp (neither
            # tensor is read before the first eviction/bit-stuff)
            bs_sb = constp.tile([128, 2], f32)
            bias_sb = bs_sb[:, 0:1]
            shift_sb = bs_sb[:, 1:2]
            hpatm_sb = constp.tile([128, NWB * H + 1], u32)
            hpat_sb = hpatm_sb[:, 0:NWB * H]
            maskc_sb = hpatm_sb[:, NWB * H:NWB * H + 1]

            if pe_warmup:
                warm = warmp.tile([128, 128], f32, tag="warm")
                for _ in range(pe_warmup):
                    nc.tensor.transpose(warm[:], ident_sb[:, :],
                                        ident_sb[:, :])

            state = {}

            def st_conv(s):
                # load x[s]=[200,32] directly transposed: one DMA per wb
                # pulls x8[w8, h] = x[h, 8wb+w8] (partition stride 1 in
                # HBM, free stride 32 — a legal 3-dim DMA AP). Costs
                # ~500ns/DMA on the underused DMA track but saves the
                # PE transposes and an ACT PSUM->SBUF copy.
                x8 = xt3p.tile([8, NWB * H], f32)
                for wb in range(NWB):
                    nc.sync.dma_start(
                        x8[:, H * wb:H * (wb + 1)],
                        x[s, :, 8 * wb:8 * wb + 8]
                        .rearrange("h w8 -> w8 h"),
                    )
                if s == 0:
                    # deferred const loads (see tile declarations above)
                    nc.sync.dma_start(bs_sb[:], bs[:])
                    nc.sync.dma_start(hpatm_sb[:], hpat[:])

                # conv per wb: 3 accumulating matmuls (taps), K=8;
                # per-tap column ranges make the H-boundary exact (no pad).
                # wb pairs share one [128, 2H] PSUM tile so eviction is 2
                # batched ACT ops instead of 4
                zs = zsp.tile([128, NWB * H], f32)
                for wb in range(NWB):
                    if wb % 2 == 0:
                        zpair = zpsum.tile([128, 2 * H], f32)
                    z = zpair[:, (wb % 2) * H:(wb % 2 + 1) * H]
                    xv = x8[:, H * wb:H * (wb + 1)]
                    nc.tensor.matmul(z[:, 0:H], wk_sb[1],
                                     xv[:, 0:H], start=True, stop=False)
                    nc.tensor.matmul(z[:, 1:H], wk_sb[0],
                                     xv[:, 0:H - 1], start=False, stop=False)
                    nc.tensor.matmul(z[:, 0:H - 1], wk_sb[2],
                                     xv[:, 1:H], start=False, stop=True)
                    # eviction as Relu(z + C_o): the per-channel shift C_o
                    # exceeds max|z| so nothing clamps and all candidates
                    # are strictly positive — threshold masking can then
                    # never collide with a live candidate. The tanh bias
                    # is b_o - C_o so the output is exact.
                    if wb % 2 == 1:
                        nc.scalar.activation(
                            zs[:, H * (wb - 1):H * (wb + 1)], zpair[:],
                            mybir.ActivationFunctionType.Relu,
                            bias=shift_sb)
                # bit-stuff the low 8 mantissa bits with the column index:
                # zs = (zs & 0xFFFFFF00) | h. Makes every value in a row
                # strictly distinct (distortion <= v*3e-5), so the Sign
                # threshold masking hits exactly the extracted instances —
                # it is otherwise broken by exact f32 duplicates at round
                # boundaries (PE fp32 conv produces them at ~1e-4/row)
                zu = zs[:].bitcast(u32)
                stuff_eng = nc.gpsimd if stuff_engine == "pool" else nc.vector
                stuff_eng.scalar_tensor_tensor(
                    zu, zu, maskc_sb, hpat_sb,
                    mybir.AluOpType.bitwise_and, mybir.AluOpType.bitwise_or)
                vt = vp.tile([128, NWB * NV], f32, tag="vt")
                state[s] = (zs, vt)

            def st_mask(s, r):
                zs, vt = state[s]
                zsl = [zs[:, H * wb:H * (wb + 1)] for wb in range(NWB)]
                other = [wb for wb in range(NWB) if (r, wb) not in mr_slots]
                for wb in range(NWB):
                    if (r, wb) in mr_slots:
                        # exact-8 removal on the DVE
                        nc.vector.match_replace(
                            zsl[wb],
                            vt[:, wb * NV + 8 * r - 8:wb * NV + 8 * r],
                            zsl[wb], MR_FILL)
                if other:
                    m = mkp.tile([128, NWB * H], f32, tag="mk")
                    for wb in other:
                        vlast = vt[:, wb * NV + 8 * r - 1:wb * NV + 8 * r]
                        if (r, wb) in pool_cmp_slots:
                            # 0/1 mask on Pool: is_lt(z, v_last)
                            nc.gpsimd.tensor_scalar(
                                m[:, H * wb:H * (wb + 1)], zsl[wb],
                                vlast, None, mybir.AluOpType.is_lt)
                        else:
                            # -1/0/+1 mask on ACT: Sign(v_last - z)
                            nc.scalar.activation(
                                m[:, H * wb:H * (wb + 1)], zsl[wb],
                                mybir.ActivationFunctionType.Sign,
                                bias=vlast, scale=-1.0)
                    lo, hi = min(other), max(other)
                    if dma_apply:
                        nc.gpsimd.dma_start(
                            zs[:, H * lo:H * (hi + 1)],
                            m[:, H * lo:H * (hi + 1)],
                            accum_op=mybir.AluOpType.mult)
                    else:
                        nc.gpsimd.tensor_tensor(
                            zs[:, H * lo:H * (hi + 1)],
                            m[:, H * lo:H * (hi + 1)],
                            zs[:, H * lo:H * (hi + 1)],
                            mybir.AluOpType.mult)

            def st_max(s, r):
                zs, vt = state[s]
                for wb in range(NWB):
                    nc.vector.max(vt[:, wb * NV + 8 * r:wb * NV + 8 * r + 8],
                                  zs[:, H * wb:H * (wb + 1)])

            def st_out(s):
                _, vt = state.pop(s)
                # one batched tanh(top50 + bias) over all 4 wb via a
                # strided view of vt (stride NV=56, take 50 per block)
                res = resp.tile([128, NWB * TOPK], f32, tag="res")
                nc.scalar.activation(
                    res[:].rearrange("p (wb r) -> p wb r", wb=NWB),
                    vt[:].rearrange("p (wb v) -> p wb v", wb=NWB)[:, :, 0:TOPK],
                    mybir.ActivationFunctionType.Tanh,
                    bias=bias_sb)
                # transpose [(o,w8), (wb2,r)] -> [(wb2,r), (o,w8)] in two
                # 100-column halves written side by side into ONE PSUM
                # tile, so the PSUM->SBUF eviction is a single ACT copy;
                # then per-wb DMA (DRAM side of one wb is a 3-dim AP)
                oT = otpsum.tile([100, 256], f32, tag="oT")
                nc.tensor.transpose(oT[:, 0:128], res[:, 0:100],
                                    ident_sb[:, :])
                nc.tensor.transpose(oT[:, 128:256], res[:, 100:200],
                                    ident_sb[:, :])
                u2 = up.tile([100, 256], f32, tag="u2")
                nc.scalar.copy(u2[:], oT[:])
                for hh in range(2):
                    for j in range(2):
                        wb = 2 * hh + j
                        nc.sync.dma_start(
                            out[s, :, :, 8 * wb:8 * wb + 8]
                            .rearrange("o r w8 -> r o w8"),
                            u2[50 * j:50 * j + 50, 128 * hh:128 * hh + 128]
                            .rearrange("r (o w8) -> r o w8", o=COUT),
                        )

            # software pipeline: 15 stages per sample (conv, max0,
            # [mask r, max r] for r=1..6, output), emitted diagonally so
            # each engine's in-order stream interleaves ~15 samples and
            # every cross-engine mask hop gets a full slot of slack
            stages = ([st_conv, lambda s: st_max(s, 0)] + [
                st for r in range(1, 7) for st in (
                    (lambda rr: (lambda s: st_mask(s, rr)))(r),
                    (lambda rr: (lambda s: st_max(s, rr)))(r),
                )
            ] + [st_out])
            nst = len(stages)
            for t in range(n_samples + nst - 1):
                for k in reversed(range(nst)):
                    s = t - k
                    if 0 <= s < n_samples:
                        stages[k](s)

    nc.compile()
    return nc


def _prep_consts(conv_w, conv_b):
    conv_w = np.asarray(conv_w, dtype=np.float32)
    conv_b = np.asarray(conv_b, dtype=np.float32)
    wmat = conv_w[:, 0, :, 0]  # [COUT, KH]
    wkj = np.zeros((KH, 8, 128), dtype=np.float32)
    for k in range(KH):
        for o in range(COUT):
            for w8 in range(8):
                wkj[k, w8, o * 8 + w8] = wmat[o, k]
    # per-channel positivity shift: C_o > max|z| guarantees z + C_o > 0
    # (|z| <= ||w_o||_1 * max|x|, and max|x| < 8 for any gaussian input)
    c_o = 8.0 * np.abs(wmat).sum(axis=1) + 1e-3  # [COUT]
    shift_p = np.repeat(c_o, 8).astype(np.float32)[:, None]  # [(o,w8), 1]
    bias_p = np.repeat(conv_b - c_o, 8).astype(np.float32)[:, None]
    ident = np.eye(128, dtype=np.float32)
    # bias/shift packed [128,2]; hpat with the stuff mask in column 800;
    # the 3 weight taps side by side in the free dim ([8, 3*128]) so one
    # DMA loads them and each tap slice starts at partition base 0
    bs_p = np.concatenate([bias_p, shift_p], axis=1).astype(np.float32)
    hpatm = np.empty((128, NWB * H + 1), dtype=np.uint32)
    hpatm[:, :NWB * H] = (np.arange(NWB * H, dtype=np.uint32) % H)[None, :]
    hpatm[:, NWB * H] = 0xFFFFFF00
    wk8 = wkj.transpose(1, 0, 2).reshape(8, KH * 128)
    return np.ascontiguousarray(wk8), bs_p, ident, hpatm


def get_compiled(n_samples=BS, mr_slots=MR_SLOTS,
                 pool_cmp_slots=POOL_CMP_SLOTS, stuff_engine=STUFF_ENGINE,
                 dma_apply=DMA_APPLY):
    key = (n_samples, tuple(mr_slots), tuple(pool_cmp_slots), stuff_engine,
           dma_apply)
    if key not in _CACHE:
        _CACHE[key] = build_module(n_samples, mr_slots=mr_slots,
                                   pool_cmp_slots=pool_cmp_slots,
                                   stuff_engine=stuff_engine,
                                   dma_apply=dma_apply)
    return _CACHE[key]


def _make_runner(nc):
    """Build a reusable jitted SPMD executor (jit traced once, reused across
    kernel() calls — run_bass_kernel_spmd re-traces on every call)."""
    import jax
    from jax.sharding import Mesh, PartitionSpec
    from jax.experimental.shard_map import shard_map
    from concourse import mybir
    from concourse.bass2jax import (_bass_exec_p, install_neuronx_cc_hook,
                                    partition_id_tensor)

    install_neuronx_cc_hook()
    in_names, out_names, out_avals, out_shapes = [], [], [], []
    pid = nc.partition_id_tensor.name if nc.partition_id_tensor else None
    for alloc in nc.m.functions[0].allocations:
        if not isinstance(alloc, mybir.MemoryLocationSet):
            continue
        name = alloc.memorylocations[0].name
        if alloc.kind == "ExternalInput":
            if name != pid:
                in_names.append(name)
        elif alloc.kind == "ExternalOutput":
            out_names.append(name)
            shape = tuple(alloc.tensor_shape)
            dtype = mybir.dt.np(alloc.dtype)
            out_avals.append(jax.core.ShapedArray(shape, dtype))
            out_shapes.append((shape, dtype))
    n_params = len(in_names)
    all_in = in_names + out_names + ([pid] if pid else [])

    def _body(*args):
        operands = list(args)
        if pid is not None:
            operands.append(partition_id_tensor())
        return tuple(_bass_exec_p.bind(
            *operands, out_avals=tuple(out_avals), in_names=tuple(all_in),
            out_names=tuple(out_names), lowering_input_output_aliases=(),
            sim_require_finite=True, sim_require_nnan=True, nc=nc))

    devices = jax.devices()[:N_CORES]
    assert len(devices) == N_CORES
    mesh = Mesh(np.asarray(devices), ("core",))
    nio = n_params + len(out_names)
    sharded = jax.jit(
        shard_map(_body, mesh=mesh,
                  in_specs=(PartitionSpec("core"),) * nio,
                  out_specs=(PartitionSpec("core"),) * len(out_names),
                  check_rep=False),
        donate_argnums=tuple(range(n_params, nio)), keep_unused=True)

    import jax.numpy as jnp
    from jax.sharding import NamedSharding
    shd = NamedSharding(mesh, PartitionSpec("core"))
    make_zeros = jax.jit(
        lambda: tuple(jnp.zeros((N_CORES * s[0],) + s[1:], d)
                      for s, d in out_shapes),
        out_shardings=(shd,) * len(out_shapes))

    def run(global_inputs):
        ins = [global_inputs[n] for n in in_names]
        zeros = jax.block_until_ready(make_zeros())
        outs = jax.block_until_ready(sharded(*ins, *zeros))
        return {n: np.asarray(o) for n, o in zip(out_names, outs)}

    return run


def kernel(x, conv_w, conv_b):
    x = np.asarray(x, dtype=np.float32)
    nc = get_compiled(BS)
    wkj, bs_p, ident, hpatm = _prep_consts(conv_w, conv_b)
    xs = np.ascontiguousarray(x.reshape(B, H, W))  # squeeze CIN=1

    if "runner" not in _CACHE:
        try:
            _CACHE["runner"] = _make_runner(nc)
        except Exception:
            _CACHE["runner"] = None
    runner = _CACHE["runner"]

    if runner is not None:
        global_inputs = {
            "x": xs,
            "wkj": np.concatenate([wkj] * N_CORES, axis=0),
            "bs_p": np.concatenate([bs_p] * N_CORES, axis=0),
            "ident": np.concatenate([ident] * N_CORES, axis=0),
            "hpatm": np.concatenate([hpatm] * N_CORES, axis=0),
        }
        # the axon terminal occasionally throws a transient
        # NRT_EXEC_UNIT_UNRECOVERABLE; a retry on a fresh executable
        # succeeds, so retry before giving up on the fast path
        for attempt in range(3):
            try:
                out = runner(global_inputs)["out"]
                return out.reshape(B, COUT, TOPK, W)
            except Exception:
                import time as _time
                _time.sleep(2.0 * (attempt + 1))
                try:
                    runner = _make_runner(nc)
                    _CACHE["runner"] = runner
                except Exception:
                    break

    # fallback: stock SPMD path (re-traces jit per call)
    from concourse.bass_utils import run_bass_kernel_spmd
    in_maps = []
    for c in range(N_CORES):
        in_maps.append({
            "x": np.ascontiguousarray(xs[c * BS:(c + 1) * BS]),
            "wkj": wkj,
            "bs_p": bs_p,
            "ident": ident,
            "hpatm": hpatm,
        })
    last_err = None
    for attempt in range(3):
        try:
            res = run_bass_kernel_spmd(nc, in_maps, list(range(N_CORES)))
            return np.concatenate(
                [res.results[c]["out"] for c in range(N_CORES)], axis=0)
        except Exception as e:
            last_err = e
            import time as _time
            _time.sleep(2.0 * (attempt + 1))
    raise last_err

